# revision 1
# baseline (speedup 1.0000x reference)
"""NT-Xent (SimCLR) contrastive loss on 8 Trainium2 NeuronCores.

Math: with x = row-normalized representation [8192, 256], tau = 0.5,
  sim = x @ x.T
  loss = (1/8192) * sum_i [ ln(sum_{j != i} exp(sim[i,j]/tau)) - sim[i, pos(i)]/tau ]
where pos(i) = (i + 4096) mod 8192.

Sharding: data-parallel over rows. Core c owns rows [c*1024, (c+1)*1024).
Each core receives the full representation (to build the transposed,
normalized key matrix xT in bf16), plus its own row slab and the partner
rows (i+4096 mod 8192) as separate inputs, so the single SPMD NEFF needs
no per-core control flow. Each core computes its [1024, 8192] similarity
slab with bf16 matmuls, exp+row-sum on the scalar engine (accum_out),
and the positive/diagonal terms via fp32 row-major dot products. The
per-row losses [128, 8] are DMA'd out; the host sums the 8 partials.

xT is stored as 16 column-chunk tiles of [128, 512] per K-half so phase-2
matmuls on chunk j can start as soon as its 4 source row-tiles are
transposed, overlapping with the rest of phase 1.
"""

import numpy as np
import ml_dtypes

import concourse.bacc as bacc
import concourse.bass as bass
import concourse.tile as tile
from concourse import mybir
from concourse.bass_utils import run_bass_kernel_spmd

N2 = 8192            # total rows (2N)
D = 256              # feature dim
NCORES = 8
ROWS = N2 // NCORES  # 1024 rows per core
N = N2 // 2          # positive-pair offset
P = 128              # SBUF partitions
KC = D // P          # 2 contraction chunks of 128
T_FULL = N2 // P     # 64 row tiles of the full matrix
T_SLAB = ROWS // P   # 8 row tiles of the slab
CCH = 512            # xT column-chunk width (= max matmul moving free)
NJ = N2 // CCH       # 16 column chunks
ACH = 1024           # activation chunk width (2 PSUM banks)
NJ2 = N2 // ACH      # 8 exp/rowsum chunks

F32 = mybir.dt.float32
BF16 = mybir.dt.bfloat16
AF = mybir.ActivationFunctionType
ALU = mybir.AluOpType


def _build_kernel(tc: tile.TileContext, out_ap, rep, slab, partner, ident_in):
    nc = tc.nc
    with (
        tc.tile_pool(name="const", bufs=1) as const,
        tc.tile_pool(name="persist", bufs=1) as persist,
        tc.tile_pool(name="work", bufs=4) as work,
        tc.tile_pool(name="small", bufs=4) as small,
        tc.tile_pool(name="exps", bufs=4) as exps,
        tc.tile_pool(name="pst", bufs=2, space="PSUM") as pst,
        tc.tile_pool(name="psmm", bufs=3, space="PSUM") as psmm,
    ):
        ident = const.tile([P, P], BF16, name="ident")
        nc.sync.dma_start(out=ident, in_=ident_in)
        ln2 = const.tile([P, 1], F32, name="ln2")
        nc.vector.memset(ln2, 0.6931471805599453)

        # persistent state
        xTc = [[persist.tile([P, CCH], BF16, tag=f"xT{k}_{j}", name=f"xT{k}_{j}")
                for j in range(NJ)] for k in range(KC)]
        sT = [persist.tile([P, ROWS], BF16, tag=f"sT{k}", name=f"sT{k}")
              for k in range(KC)]
        rsums = [persist.tile([P, NJ2], F32, tag=f"rs{m}", name=f"rs{m}")
                 for m in range(T_SLAB)]
        d_all = persist.tile([P, T_SLAB], F32, tag="d_all", name="d_all")
        pos2 = persist.tile([P, T_SLAB], F32, tag="pos2", name="pos2")
        sxm = persist.tile([P, T_SLAB], F32, tag="sxm", name="sxm")
        lossm = persist.tile([P, T_SLAB], F32, tag="lossm", name="lossm")

        def load_norm(src, t, out_dt, tag, logbias=None):
            """DMA row-tile t of src; return (x * exp(-0.5*ln||x||^2 + logbias),
            raw x, inv scale). logbias=ln(2) yields rows scaled by 2/||row||.
            rsqrt is computed as exp(-0.5*ln(ssq)) -- the InstReciprocal and
            tensor_scalar-with-AP-scalar paths abort on this runtime.
            """
            x = work.tile([P, D], F32, tag=f"ld_{tag}", name=f"ld_{tag}")
            nc.sync.dma_start(out=x, in_=src[t * P:(t + 1) * P, :])
            sq = work.tile([P, D], F32, tag="sq", name="sq")
            nc.vector.tensor_mul(sq, x, x)
            ssq = small.tile([P, 1], F32, tag="ssq", name="ssq")
            nc.vector.reduce_sum(ssq, sq, axis=mybir.AxisListType.X)
            lssq = small.tile([P, 1], F32, tag="lssq", name="lssq")
            nc.scalar.activation(lssq, ssq, AF.Ln)
            inv = small.tile([P, 1], F32, tag="inv", name="inv")
            nc.scalar.activation(inv, lssq, AF.Exp, scale=-0.5,
                                 bias=0.0 if logbias is None else logbias)
            xn = work.tile([P, D], out_dt, tag=f"xn_{tag}", name=f"xn_{tag}")
            nc.scalar.activation(xn, x, AF.Copy, scale=inv)
            return xn, x, inv

        def transpose_tile(xb, put):
            """xb [128, 256] bf16; put(k, psum_tile) stores the k-th half."""
            for k in range(KC):
                pt = pst.tile([P, P], BF16, tag="pt", name="pt")
                nc.tensor.transpose(pt, xb[:, k * P:(k + 1) * P], ident)
                put(k, pt)

        # phase 1a: slab rows -> sT (bf16 queries) + d_i; partner -> pos2_i
        for t in range(T_SLAB):
            xs, xraw, inv = load_norm(slab, t, F32, "slab")
            xsb = work.tile([P, D], BF16, tag="xsb", name="xsb")
            nc.scalar.activation(xsb, xraw, AF.Copy, scale=inv)
            transpose_tile(
                xsb, lambda k, pt, t=t: nc.vector.tensor_copy(
                    sT[k][:, t * P:(t + 1) * P], pt))
            sq2 = work.tile([P, D], F32, tag="sq2", name="sq2")
            nc.vector.tensor_mul(sq2, xs, xs)
            nc.vector.reduce_sum(d_all[:, t:t + 1], sq2,
                                 axis=mybir.AxisListType.X)
            xp, _, _ = load_norm(partner, t, F32, "part", logbias=ln2)
            sq3 = work.tile([P, D], F32, tag="sq3", name="sq3")
            nc.vector.tensor_mul(sq3, xs, xp)
            nc.vector.reduce_sum(pos2[:, t:t + 1], sq3,
                                 axis=mybir.AxisListType.X)

        # phase 1b: full rep -> normalized, transposed key chunks xTc (bf16)
        for t in range(T_FULL):
            xb, _, _ = load_norm(rep, t, BF16, "full")
            j, off = divmod(t * P, CCH)
            transpose_tile(
                xb, lambda k, pt, j=j, off=off: nc.vector.tensor_copy(
                    xTc[k][j][:, off:off + P], pt))

        # phase 2: similarity slab in [128, 1024] chunks; exp + row sums.
        # j2-outer so chunk j2 only needs xTc[:][2*j2:2*j2+2] (overlaps ph1b).
        for j2 in range(NJ2):
            for m in range(T_SLAB):
                ps = psmm.tile([P, ACH], F32, tag="ps", name="ps")
                for half in range(2):
                    jj = 2 * j2 + half
                    for k in range(KC):
                        nc.tensor.matmul(
                            ps[:, half * CCH:(half + 1) * CCH],
                            sT[k][:, m * P:(m + 1) * P],
                            xTc[k][jj],
                            start=(k == 0), stop=(k == KC - 1))
                esc = exps.tile([P, ACH], BF16, tag="esc", name="esc")
                nc.scalar.activation(esc, ps, AF.Exp, scale=2.0,
                                     accum_out=rsums[m][:, j2:j2 + 1])

        # tails: S_m - exp(2 d_m), then one Ln + subtract over all columns
        for m in range(T_SLAB):
            S = small.tile([P, 1], F32, tag="S", name="S")
            nc.vector.reduce_sum(S, rsums[m], axis=mybir.AxisListType.X)
            ed = small.tile([P, 1], F32, tag="ed", name="ed")
            nc.scalar.activation(ed, d_all[:, m:m + 1], AF.Exp, scale=2.0)
            nc.vector.tensor_sub(sxm[:, m:m + 1], S, ed)
        nc.scalar.activation(lossm, sxm, AF.Ln)
        nc.vector.tensor_sub(lossm, lossm, pos2)
        nc.sync.dma_start(out=out_ap, in_=lossm)


def build_nc():
    nc = bacc.Bacc("TRN2", target_bir_lowering=False, debug=False,
                   num_devices=NCORES)
    rep = nc.dram_tensor("rep", [N2, D], F32, kind="ExternalInput").ap()
    slab = nc.dram_tensor("slab", [ROWS, D], F32, kind="ExternalInput").ap()
    partner = nc.dram_tensor("partner", [ROWS, D], F32,
                             kind="ExternalInput").ap()
    ident_in = nc.dram_tensor("ident", [P, P], BF16,
                              kind="ExternalInput").ap()
    out = nc.dram_tensor("out", [P, T_SLAB], F32, kind="ExternalOutput").ap()
    with tile.TileContext(nc) as tc:
        _build_kernel(tc, out, rep, slab, partner, ident_in)
    nc.compile()
    return nc


_NC = None
LAST_RESULTS = None
_IDENT = np.eye(P, dtype=np.float32).astype(ml_dtypes.bfloat16)


def _make_in_maps(rep: np.ndarray):
    in_maps = []
    for c in range(NCORES):
        r0 = c * ROWS
        slab = np.ascontiguousarray(rep[r0:r0 + ROWS])
        pidx = (np.arange(r0, r0 + ROWS) + N) % N2
        partner = np.ascontiguousarray(rep[pidx])
        in_maps.append({"rep": rep, "slab": slab, "partner": partner,
                        "ident": _IDENT})
    return in_maps


def kernel(representation: np.ndarray, **run_kwargs) -> np.ndarray:
    global _NC, LAST_RESULTS
    rep = np.ascontiguousarray(np.asarray(representation), dtype=np.float32)
    assert rep.shape == (N2, D)
    if _NC is None:
        _NC = build_nc()
    res = run_bass_kernel_spmd(_NC, _make_in_maps(rep),
                               core_ids=list(range(NCORES)), **run_kwargs)
    LAST_RESULTS = res
    total = 0.0
    for r in res.results:
        total += float(r["out"].astype(np.float64).sum())
    return np.asarray(np.float32(total / N2))



# revision 2
# speedup vs baseline: 4.1945x; 4.1945x over previous
"""NT-Xent (SimCLR) contrastive loss on 8 Trainium2 NeuronCores.

Math: with x = row-normalized representation [8192, 256], tau = 0.5,
  sim = x @ x.T
  loss = (1/8192) * sum_i [ ln(sum_{j != i} exp(2 sim[i,j])) - 2 sim[i, pos(i)] ]
where pos(i) = (i + 4096) mod 8192.

Split of work:
  Host (O(N*D), numpy): row-normalize, cast to bf16, build per-core
  row-rolled copies, positive-pair dot products, final ln/reduction.
  Device (O(N^2*D)): the 8192x8192 similarity matrix and the row sums of
  exp(2*sim). Core c computes rows [1024c, 1024c+1024) of sim against all
  8192 columns and returns the per-row partial sums of exp.

Device kernel (SPMD, identical program on 8 cores; each core gets x rolled
so its 1024 slab rows sit at rows 0..1023):
  1. 16 xbar transpose-DMAs (DRAM bf16 -> SBUF) build xT[k] = x[:, 128k:128k+128].T
     as 8 tiles of [128, 1024] per k-half. Slab tiles (j=0) land first.
  2. For each of 4 column chunks of 2048 and 8 slab row-tiles: 8 matmuls
     (bf16, N=512, K=2x128) accumulate sim into a [128, 2048] PSUM tile
     (2-buffer ping-pong = all 8 banks).
  3. One scalar-engine Exp (scale=2.0) per PSUM tile with accum_out giving
     the per-partition row sum; 32 sums -> rs [128, 32] f32, DMA'd out.

Host then: S_i = sum of the 4 chunk partials, denom = S - exp(2*s_ii_bf16),
loss = mean(ln(denom) - 2*pos).
"""

import numpy as np
import ml_dtypes

import concourse.bacc as bacc
import concourse.tile as tile
from concourse import mybir
from concourse.bass_utils import run_bass_kernel_spmd

N2 = 8192            # total rows (2N)
D = 256              # feature dim
NCORES = 8
ROWS = N2 // NCORES  # 1024 slab rows per core
N = N2 // 2          # positive-pair offset
P = 128              # SBUF partitions
KC = D // P          # 2 contraction chunks of 128
CHUNK = 1024         # xT tile width (rows of x per transpose DMA)
NT = N2 // CHUNK     # 8 xT tiles per k-half
J2W = 2048           # psum/exp chunk width (4 PSUM banks)
NJ2 = N2 // J2W      # 4 exp chunks per slab row-tile
MT = ROWS // P       # 8 slab row-tiles

F32 = mybir.dt.float32
BF16 = mybir.dt.bfloat16
AF = mybir.ActivationFunctionType


def _build_kernel(tc: tile.TileContext, rs_out, xk):
    nc = tc.nc
    with (
        tc.tile_pool(name="xt", bufs=1) as xtp,
        tc.tile_pool(name="acc", bufs=1) as accp,
        tc.tile_pool(name="esc", bufs=2) as escp,
        tc.tile_pool(name="ps", bufs=2, space="PSUM") as psp,
    ):
        xts = [[xtp.tile([P, CHUNK], BF16, tag=f"xt{k}_{j}", name=f"xt{k}_{j}")
                for j in range(NT)] for k in range(KC)]
        rs = accp.tile([P, NJ2 * MT], F32, tag="rs", name="rs")

        # transposed, normalized bf16 keys; slab tiles (j=0) first
        for j in range(NT):
            for k in range(KC):
                nc.sync.dma_start(
                    out=xts[k][j],
                    in_=xk[k][j * CHUNK:(j + 1) * CHUNK, :],
                    transpose=True)

        # similarity chunks + exp row-sums
        for j2 in range(NJ2):
            for m in range(MT):
                ps = psp.tile([P, J2W], F32, tag="ps", name="ps")
                for half in range(J2W // 512):
                    col = j2 * J2W + half * 512
                    jj, off = divmod(col, CHUNK)
                    for k in range(KC):
                        nc.tensor.matmul(
                            ps[:, half * 512:(half + 1) * 512],
                            xts[k][0][:, m * P:(m + 1) * P],
                            xts[k][jj][:, off:off + 512],
                            start=(k == 0), stop=(k == KC - 1))
                esc = escp.tile([P, J2W], BF16, tag="esc", name="esc")
                nc.scalar.activation(esc, ps, AF.Exp, scale=2.0,
                                     accum_out=rs[:, j2 * MT + m:j2 * MT + m + 1])

        nc.sync.dma_start(out=rs_out, in_=rs)


def build_nc():
    nc = bacc.Bacc("TRN2", target_bir_lowering=False, debug=False,
                   num_devices=NCORES)
    xk = [nc.dram_tensor(f"x{k}", [N2, P], BF16, kind="ExternalInput").ap()
          for k in range(KC)]
    rs_out = nc.dram_tensor("rs", [P, NJ2 * MT], F32,
                            kind="ExternalOutput").ap()
    with tile.TileContext(nc) as tc:
        _build_kernel(tc, rs_out, xk)
    nc.compile()
    return nc


_NC = None
LAST_RESULTS = None


def _make_in_maps(xb16: np.ndarray):
    in_maps = []
    for c in range(NCORES):
        xr = np.roll(xb16, -c * ROWS, axis=0)
        in_maps.append({f"x{k}": np.ascontiguousarray(xr[:, k * P:(k + 1) * P])
                        for k in range(KC)})
    return in_maps


def kernel(representation: np.ndarray, **run_kwargs) -> np.ndarray:
    global _NC, LAST_RESULTS
    rep = np.asarray(representation, dtype=np.float32)
    assert rep.shape == (N2, D)

    # host prep: normalize (f32, matching torch CosineSimilarity eps), bf16
    norms = np.maximum(np.sqrt((rep.astype(np.float64) ** 2).sum(axis=1)),
                       1e-8)
    xn = (rep / norms[:, None]).astype(np.float32)
    xb16 = xn.astype(ml_dtypes.bfloat16)

    if _NC is None:
        _NC = build_nc()
    res = run_bass_kernel_spmd(_NC, _make_in_maps(xb16),
                               core_ids=list(range(NCORES)), **run_kwargs)
    LAST_RESULTS = res

    # gather: S_i = sum over the 4 column-chunk partials of exp(2 sim)
    S = np.empty(N2, dtype=np.float64)
    for c, r in enumerate(res.results):
        rs = r["rs"].astype(np.float64).reshape(P, NJ2, MT)  # [p, j2, m]
        part = rs.sum(axis=1)                                # [p, m]
        S[c * ROWS:(c + 1) * ROWS] = part.T.reshape(ROWS)    # m*128 + p

    # host tail: remove diagonal (as the device computed it, i.e. from bf16
    # inputs), add positive terms, final log/mean
    xb = xb16.astype(np.float64)
    ssb = (xb * xb).sum(axis=1)                  # device's sim[i,i]
    denom = S - np.exp(2.0 * ssb)
    xn64 = xn.astype(np.float64)
    pos = (xn64 * np.roll(xn64, -N, axis=0)).sum(axis=1)
    loss = (np.log(denom) - 2.0 * pos).mean()
    return np.asarray(np.float32(loss))


# revision 4
# speedup vs baseline: 4.5402x; 1.0824x over previous
"""NT-Xent (SimCLR) contrastive loss on 8 Trainium2 NeuronCores.

Math: with x = row-normalized representation [8192, 256], tau = 0.5,
  sim = x @ x.T
  loss = (1/8192) * sum_i [ ln(sum_{j != i} exp(2 sim[i,j])) - 2 sim[i, pos(i)] ]
where pos(i) = (i + 4096) mod 8192.

Split of work:
  Host (O(N*D), numpy): row-normalize, cast to bf16, build per-core
  row-rolled copies, positive-pair dot products, final combine/ln/mean.
  Device (O(N^2*D)): the similarity matrix and row/column sums of exp(2 sim).

Symmetry: sim is symmetric, so only ~5/8 of it is computed. Each core
receives x rolled so its 1024 slab rows sit at rows 0..1023, and computes
sim[0:1024, 0:5120] (its slab rows against column blocks b=0..4; the
b=4 block is computed by both members of a (c, c+4) pair). Row sums of
exp(2 sim) cover column blocks 0..4; the missing blocks 5..7 are the
transposes of blocks 1..3 of three other cores, recovered from COLUMN
sums of exp over blocks 1..3 (computed on-device with ones-stationary
matmuls accumulating in PSUM across the 8 row-tiles). The host combines
row + column partials into full row sums S_i.

Device kernel (SPMD, identical program on all 8 cores):
  1. 10 xbar transpose-DMAs (DRAM bf16 -> SBUF) build xT chunks
     [128, 1024] for columns 0..5120 (slab tiles first).
  2. Per slab row-tile m (8): column chunks {1536,1536,1536,512} ->
     [128,1536] PSUM (2-buffer ping-pong, 6 banks) via bf16 matmuls
     (N=512, K=2x128); scalar Exp (scale=2) per chunk writes bf16 to
     SBUF with accum_out row partials -> rs [128, 32].
  3. Column sums: six ones-stationary matmuls per m over the exp'd
     [128,512] slices of blocks 1..3, accumulated over m in two PSUM
     banks at partitions {0,32,64}; emitted one m behind the main
     matmuls to keep the PE stream dense (HAM stays warm). DVE copies
     the two banks to SBUF at the end; DMA'd out with rs.
"""

import numpy as np
import ml_dtypes

import concourse.bacc as bacc
import concourse.tile as tile
from concourse import mybir
from concourse.bass_utils import run_bass_kernel_spmd

N2 = 8192            # total rows (2N)
D = 256              # feature dim
NCORES = 8
ROWS = N2 // NCORES  # 1024 slab rows per core
N = N2 // 2          # positive-pair offset
P = 128              # SBUF partitions
KC = D // P          # 2 contraction chunks of 128
CHUNK = 1024         # xT tile width
NXT = 5              # xT tiles per k-half (cols 0..5120)
COLS = NXT * CHUNK   # 5120 columns computed per slab row-tile
MT = ROWS // P       # 8 slab row-tiles
CW = [1536, 1536, 1536, 512]   # exp chunk widths per m (sum = COLS)
CSTART = [0, 1536, 3072, 4608]
NCH = len(CW)
# column-sum slices: six 512-wide slices covering rolled cols [1024, 4096)
# as (chunk index, offset within chunk); slice i covers cols 1024+512*i
RED = [(0, 1024), (1, 0), (1, 512), (1, 1024), (2, 0), (2, 512)]

F32 = mybir.dt.float32
BF16 = mybir.dt.bfloat16
AF = mybir.ActivationFunctionType


def _build_kernel(tc: tile.TileContext, rs_out, cols_out, xk):
    nc = tc.nc
    with (
        tc.tile_pool(name="xt", bufs=1) as xtp,
        tc.tile_pool(name="acc", bufs=1) as accp,
        tc.tile_pool(name="esc", bufs=8) as escp,
        tc.tile_pool(name="ps", bufs=2, space="PSUM") as psp,
        tc.tile_pool(name="red", bufs=1, space="PSUM") as redp,
    ):
        xts = [[xtp.tile([P, CHUNK], BF16, tag=f"xt{k}_{j}", name=f"xt{k}_{j}")
                for j in range(NXT)] for k in range(KC)]
        rs = accp.tile([P, NCH * MT], F32, tag="rs", name="rs")
        ones = accp.tile([P, 1], BF16, tag="ones", name="ones")
        colsb = accp.tile([P, 1024], F32, tag="colsb", name="colsb")
        red = [redp.tile([P, 512], F32, tag=f"red{t}", name=f"red{t}")
               for t in range(2)]
        nc.vector.memset(ones, 1.0)

        # transposed bf16 keys for columns 0..5120; slab tiles (j=0) first
        for j in range(NXT):
            for k in range(KC):
                nc.sync.dma_start(
                    out=xts[k][j],
                    in_=xk[k][j * CHUNK:(j + 1) * CHUNK, :],
                    transpose=True)

        escs = {}  # (m, c) -> exp'd chunk tile

        def red_mms(m):
            for i, (c, off) in enumerate(RED):
                t, bp = i % 2, 32 * (i // 2)
                nc.tensor.matmul(
                    red[t][bp:bp + 1, :],
                    ones,
                    escs[(m, c)][:, off:off + 512],
                    start=(m == 0), stop=(m == MT - 1),
                    skip_group_check=True)

        for m in range(MT):
            for c in range(NCH):
                w = CW[c]
                ps = psp.tile([P, 1536], F32, tag="ps", name="ps")
                for half in range(w // 512):
                    col = CSTART[c] + half * 512
                    jj, off = divmod(col, CHUNK)
                    for k in range(KC):
                        nc.tensor.matmul(
                            ps[:, half * 512:(half + 1) * 512],
                            xts[k][0][:, m * P:(m + 1) * P],
                            xts[k][jj][:, off:off + 512],
                            start=(k == 0), stop=(k == KC - 1))
                esc = escp.tile([P, 1536], BF16, tag="esc", name="esc")
                escs[(m, c)] = esc
                nc.scalar.activation(
                    esc[:, :w], ps[:, :w], AF.Exp, scale=2.0,
                    accum_out=rs[:, m * NCH + c:m * NCH + c + 1])
            # column-sum matmuls one m behind: keeps the PE queue dense
            if m > 0:
                red_mms(m - 1)
        red_mms(MT - 1)

        for t in range(2):
            nc.vector.tensor_copy(colsb[:, t * 512:(t + 1) * 512], red[t])
        nc.sync.dma_start(out=rs_out, in_=rs)
        nc.sync.dma_start(out=cols_out, in_=colsb)


def build_nc():
    nc = bacc.Bacc("TRN2", target_bir_lowering=False, debug=False,
                   num_devices=NCORES)
    xk = [nc.dram_tensor(f"x{k}", [N2, P], BF16, kind="ExternalInput").ap()
          for k in range(KC)]
    rs_out = nc.dram_tensor("rs", [P, NCH * MT], F32,
                            kind="ExternalOutput").ap()
    cols_out = nc.dram_tensor("cols", [P, 1024], F32,
                              kind="ExternalOutput").ap()
    with tile.TileContext(nc) as tc:
        _build_kernel(tc, rs_out, cols_out, xk)
    nc.compile()
    return nc


_NC = None
LAST_RESULTS = None


def _make_in_maps(xb16: np.ndarray):
    in_maps = []
    for c in range(NCORES):
        xr = np.roll(xb16, -c * ROWS, axis=0)
        in_maps.append({f"x{k}": np.ascontiguousarray(xr[:, k * P:(k + 1) * P])
                        for k in range(KC)})
    return in_maps


def kernel(representation: np.ndarray, **run_kwargs) -> np.ndarray:
    global _NC, LAST_RESULTS
    rep = np.asarray(representation, dtype=np.float32)
    assert rep.shape == (N2, D)

    # host prep: normalize (f32, matching torch CosineSimilarity eps), bf16
    norms = np.maximum(np.sqrt((rep.astype(np.float64) ** 2).sum(axis=1)),
                       1e-8)
    xn = (rep / norms[:, None]).astype(np.float32)
    xb16 = xn.astype(ml_dtypes.bfloat16)

    if _NC is None:
        _NC = build_nc()
    res = run_bass_kernel_spmd(_NC, _make_in_maps(xb16),
                               core_ids=list(range(NCORES)), **run_kwargs)
    LAST_RESULTS = res

    # combine row partials (cols 0..5120 rolled) and column partials
    # (rolled cols 1024..4096, i.e. blocks b=1..3) into full row sums S
    S = np.zeros(N2, dtype=np.float64)
    for c, r in enumerate(res.results):
        rs = r["rs"].astype(np.float64).reshape(P, MT, NCH)  # [p, m, ch]
        own = rs.sum(axis=2).T.reshape(ROWS)                 # row m*128+p
        S[c * ROWS:(c + 1) * ROWS] += own
        cols = r["cols"].astype(np.float64)                  # [p, 1024]
        for i in range(6):
            colsum = cols[32 * (i // 2), (i % 2) * 512:(i % 2) * 512 + 512]
            g0 = (c * ROWS + 1024 + 512 * i) % N2
            S[g0:g0 + 512] += colsum

    # host tail: remove diagonal (as the device computed it, from bf16
    # inputs), add positive terms, final log/mean
    xb = xb16.astype(np.float64)
    ssb = (xb * xb).sum(axis=1)                  # device's sim[i,i]
    denom = S - np.exp(2.0 * ssb)
    xn64 = xn.astype(np.float64)
    pos = (xn64 * np.roll(xn64, -N, axis=0)).sum(axis=1)
    loss = (np.log(denom) - 2.0 * pos).mean()
    return np.asarray(np.float32(loss))
